# revision 23
# baseline (speedup 1.0000x reference)
"""Causal attention (B=4, S=2048, D=1024) on 8 Trainium2 NeuronCores.

Sharding: 2 cores per batch element, query blocks of 256 rows split by parity
(fold 0 takes odd blocks, fold 1 even) so causal work balances.

Algebraic restructure vs the straightforward QKV pipeline: with
M = Wq^T Wk (precomputed host-side from the weights), scores are
S = (x_q M) x^T, so no K projection is needed on-device and "K^T" is the raw
transposed input x^T already resident in SBUF. On the output side,
out = A (x Wv^T) is re-associated as (A x) Wv^T, so no V projection either:
the attention matrix contracts against raw x (natural layout), and one final
d x d projection by Wv^T produces the output. This removes the duplicated
K/V projections entirely (they were recomputed on both cores of a pair) at
zero communication cost.

All matmuls run in bf16 (fp32 PSUM accumulate). Scores are computed
transposed (S^T = (x^T)^T-stationary @ q'^T) so exp(S^T) tiles feed the
A-x contraction directly as the moving operand, producing ax^T =
sum_k x[k,:]^T es[k,:] in [d, q] layout, which in turn is the stationary for
the final projection out[q, o] = sum_d ax^T[d, q] Wv^T[d, o] -- every tensor
lands in its natural layout with no on-chip transposes.

Causal structure: per core 4 query slots of 256 rows; slot pairs (0,1) and
(2,3) share score passes. Static kt depths per pair are (4,8) and (12,16)
(fold-0 depths; fold 1 true depths are smaller and handled by its 0/1 masks,
which also zero the diagonal/overcomputed regions and keep softmax
denominators exact). Scores/es run 512 wide for kt < d_lo (both slots) and
256 wide for the deep slot's tail. Softmax skips max-subtraction (scaled
scores are ~N(0,1); exp cannot overflow), denominators via ones-column
matmuls per 128-query chunk.
"""

import sys

sys.path.insert(0, "/opt/trn_rl_repo")

import ml_dtypes
import numpy as np

import concourse.bass as bass  # noqa: F401
import concourse.mybir as mybir
import concourse.tile as tile
from concourse import bacc
from concourse.bass_utils import run_bass_kernel_spmd

F32 = mybir.dt.float32
BF16 = mybir.dt.bfloat16
AF = mybir.ActivationFunctionType

B, S, D = 4, 2048, 1024
P = 128
DC = D // P  # 8 contraction chunks
TC = S // P  # 16 context chunks
N_CORES = 8
SLOTS = 4
QB = 256
FOLD_QBLOCKS = {0: [1, 3, 5, 7], 1: [0, 2, 4, 6]}
# Static (fold-0) kt depths for slot pairs (0,1) and (2,3).
PAIRS = [(4, 8), (12, 16)]
N_M512 = sum(dlo for dlo, _ in PAIRS)  # full-width mask tiles
N_M256 = sum(dhi - dlo for dlo, dhi in PAIRS)  # deep-tail mask tiles
SCALE = 1.0 / np.sqrt(np.float32(D))
WARMUP_MM = 6  # dummy matmuls to release the HAM clock gate early


def _build_nc(repeat: int = 1):
    nc = bacc.Bacc("TRN2", target_bir_lowering=False, debug=False, num_devices=N_CORES)

    m_d = nc.declare_dram_parameter("m", [D, D], BF16, isOutput=False)
    xqT_d = nc.declare_dram_parameter("xqT", [D, SLOTS * QB], BF16, isOutput=False)
    xT_d = nc.declare_dram_parameter("xT", [D, S], BF16, isOutput=False)
    xn_d = nc.declare_dram_parameter("xn", [S, D], BF16, isOutput=False)
    wvT_d = nc.declare_dram_parameter("wvT", [D, D], BF16, isOutput=False)
    m512_d = nc.declare_dram_parameter("m512", [N_M512, P, 512], BF16, isOutput=False)
    m256_d = nc.declare_dram_parameter("m256", [N_M256, P, 256], BF16, isOutput=False)
    out_d = nc.declare_dram_parameter("out", [SLOTS * QB, D], F32, isOutput=True)

    m_r = m_d[:].rearrange("(ic p) j -> p ic j", p=P)  # [128, 8, 1024]
    xqT_r = xqT_d[:].rearrange("(ic p) q -> p ic q", p=P)  # [128, 8, 1024]
    xT_r = xT_d[:].rearrange("(dc p) t -> p dc t", p=P)  # [128, 8, 2048]
    xn_r = xn_d[:].rearrange("(tc p) d -> p tc d", p=P)  # [128, 16, 1024]
    wvT_r = wvT_d[:].rearrange("(dc p) o -> p dc o", p=P)
    m512_r = m512_d[:].rearrange("n p w -> p n w")  # [128, 16, 512]
    m256_r = m256_d[:].rearrange("n p w -> p n w")  # [128, 8, 256]
    out_r = out_d[:].rearrange("(qc p) o -> p qc o", p=P)  # [128, 8, 1024]

    with tile.TileContext(nc, pool_alloc_mode="queue") as tc:
      for _rep in range(repeat):
        with tc.tile_pool(name="resident", bufs=1) as res_pool:
            xT_s = res_pool.tile([P, DC, S], BF16, name="xT_s")
            xn_s = res_pool.tile([P, TC, D], BF16, name="xn_s")
            # q'^T as separate (d-slice, q-half) tiles so consumers wait on
            # exactly the chunk they read, not the whole tensor.
            qpT = {
                (ds, qt): res_pool.tile([P, 512], BF16, name=f"qpT_{ds}_{qt}")
                for ds in range(DC)
                for qt in range(2)
            }
            mk512 = res_pool.tile([P, N_M512, 512], BF16, name="mk512")
            mk256 = res_pool.tile([P, N_M256, 256], BF16, name="mk256")
            ones128 = res_pool.tile([P, P], BF16, name="ones128")
            scrap = res_pool.tile([P, 512], F32, name="scrap")
            warm_rhs = res_pool.tile([P, 512], BF16, name="warm_rhs")
            nc.vector.memset(ones128[:], 1.0)
            nc.vector.memset(warm_rhs[:], 0.0)

            # ---- Phase Q': q'^T = M^T.T @ xq^T -> qpT (SBUF) ---------------
            with (
                tc.tile_pool(name="m_pool", bufs=1) as mpool,
                tc.tile_pool(name="xq_pool", bufs=1) as xqpool,
                tc.tile_pool(name="psum_q", bufs=6, space="PSUM") as psq,
                tc.tile_pool(name="psum_w", bufs=1, space="PSUM") as psw,
            ):
                m_t = {
                    (ic, h): mpool.tile([P, 512], BF16, name=f"m_{ic}_{h}")
                    for ic in range(DC)
                    for h in range(2)
                }
                xq_t = {
                    (ic, qt): xqpool.tile([P, 512], BF16, name=f"xq_{ic}_{qt}")
                    for ic in range(DC)
                    for qt in range(2)
                }
                # Warm-up: the PE clock gate (HAM) starts at 1.2 GHz and only
                # reaches 2.4 GHz after ~3.4us of sustained activity. Run a
                # dummy chain during the initial DMA prefix so the real
                # matmuls start warm.
                if WARMUP_MM:
                    ps_w = psw.tile([P, 512], F32, name="ps_warm")
                    for i in range(WARMUP_MM):
                        nc.tensor.matmul(
                            ps_w[:],
                            lhsT=ones128[:],
                            rhs=warm_rhs[:],
                            start=(i == 0),
                            stop=(i == WARMUP_MM - 1),
                        )
                    nc.vector.tensor_copy(scrap[:], ps_w[:])
                # All loads on ONE queue, in critical-path priority order:
                # the HBM pipe is the bottleneck, so later-needed tensors
                # must not steal bandwidth from the q' operands.
                for ic in range(DC):
                    nc.sync.dma_start(m_t[(ic, 0)][:], m_r[:, ic, 0:512])
                    nc.sync.dma_start(xq_t[(ic, 0)][:], xqT_r[:, ic, 0:512])
                for ic in range(DC):
                    nc.sync.dma_start(m_t[(ic, 1)][:], m_r[:, ic, 512:1024])
                    nc.sync.dma_start(xq_t[(ic, 1)][:], xqT_r[:, ic, 512:1024])
                for dc in range(DC):
                    nc.sync.dma_start(xT_s[:, dc, :], xT_r[:, dc, :])
                nc.sync.dma_start(mk512[:], m512_r)
                for tc_i in range(TC // 2):
                    nc.sync.dma_start(xn_s[:, tc_i, :], xn_r[:, tc_i, :])
                nc.sync.dma_start(mk256[:], m256_r)
                for tc_i in range(TC // 2, TC):
                    nc.sync.dma_start(xn_s[:, tc_i, :], xn_r[:, tc_i, :])
                # Chain order chases the DMA arrival order: the (h=0, qt=0)
                # operand halves land first.
                for qt in range(2):
                    for ds in range(DC):
                        ps = psq.tile([P, 512], F32, name="ps_q")
                        for ic in range(DC):
                            nc.tensor.matmul(
                                ps[:],
                                lhsT=m_t[(ic, ds // 4)][
                                    :, P * (ds % 4) : P * (ds % 4 + 1)
                                ],
                                rhs=xq_t[(ic, qt)][:],
                                start=(ic == 0),
                                stop=(ic == DC - 1),
                            )
                        nc.vector.tensor_copy(qpT[(ds, qt)][:], ps[:])

            # ---- Attention: scores -> exp/mask -> ax^T -> out projection ---
            with (
                tc.tile_pool(name="wv_pool", bufs=1) as wvpool,
                tc.tile_pool(name="es512_pool", bufs=14) as e5pool,
                tc.tile_pool(name="es256_pool", bufs=6) as e2pool,
                tc.tile_pool(name="ax_pool", bufs=2) as axpool,
                tc.tile_pool(name="ob_pool", bufs=3) as obpool,
                tc.tile_pool(name="rc_pool", bufs=2) as rcpool,
                tc.tile_pool(name="rcb_pool", bufs=2) as rcbpool,
                tc.tile_pool(name="psum_s", bufs=2, space="PSUM") as pss,
                tc.tile_pool(name="psum_a", bufs=2, space="PSUM") as psa,
                tc.tile_pool(name="psum_d", bufs=2, space="PSUM") as psd_pool,
                tc.tile_pool(name="psum_o", bufs=2, space="PSUM") as pso_pool,
            ):
                wv_s = wvpool.tile([P, DC, D], BF16, name="wv_s")
                for dc in range(DC):
                    nc.sync.dma_start(wv_s[:, dc, :], wvT_r[:, dc, :])

                i512 = 0
                i256 = 0
                for p, (dlo, dhi) in enumerate(PAIRS):
                    # scores + exp + mask over the pair's static depth
                    es_full = []
                    es_nar = []
                    for kt in range(dhi):
                        full = kt < dlo
                        w = 512 if full else 256
                        c0 = 0 if full else 256
                        ps = pss.tile([P, w], F32, name="ps_s")
                        for dc in range(DC):
                            nc.tensor.matmul(
                                ps[:],
                                lhsT=xT_s[:, dc, P * kt : P * (kt + 1)],
                                rhs=qpT[(dc, p)][:, c0 : c0 + w],
                                start=(dc == 0),
                                stop=(dc == DC - 1),
                            )
                        pool = e5pool if full else e2pool
                        es = pool.tile([P, w], BF16, name="es")
                        nc.scalar.activation(es[:], ps[:], AF.Exp, scale=SCALE)
                        if full:
                            mk = mk512[:, i512, :]
                            i512 += 1
                            es_full.append(es)
                        else:
                            mk = mk256[:, i256, :]
                            i256 += 1
                            es_nar.append(es)
                        nc.vector.tensor_mul(out=es[:], in0=es[:], in1=mk)

                    def es_cols(kt, c0, w, dlo=dlo, es_full=es_full, es_nar=es_nar):
                        """es slice for pair-local cols [c0, c0+w)."""
                        if kt < dlo:
                            return es_full[kt][:, c0 : c0 + w]
                        assert c0 >= 256
                        return es_nar[kt - dlo][:, c0 - 256 : c0 - 256 + w]

                    # denominators: all-ones [128,128] stationary replicates
                    # denom[q] = sum_k es[k, q] across every partition, so
                    # the reciprocal runs partition-parallel and the scale
                    # fuses into the ax^T PSUM->SBUF copy below.
                    rcb = {}
                    for sl, depth in ((0, dlo), (1, dhi)):
                        psd = psd_pool.tile([P, QB], F32, name="ps_d")
                        for kt in range(depth):
                            nc.tensor.matmul(
                                psd[:],
                                lhsT=ones128[:],
                                rhs=es_cols(kt, 256 * sl, 256),
                                start=(kt == 0),
                                stop=(kt == depth - 1),
                            )
                        rcb[sl] = rcbpool.tile([P, QB], F32, name="rcb")
                        nc.vector.reciprocal(rcb[sl][:], psd[:])

                    # ax^T[d, q] = (sum_k x[k, d] es[k, q]) / denom[q]
                    axT = {
                        ds: axpool.tile([P, 512], BF16, name=f"axT_{ds}")
                        for ds in range(DC)
                    }
                    for ds in range(DC):
                        for sl, depth in ((0, dlo), (1, dhi)):
                            ps = psa.tile([P, QB], F32, name="ps_a")
                            for kt in range(depth):
                                nc.tensor.matmul(
                                    ps[:],
                                    lhsT=xn_s[:, kt, P * ds : P * (ds + 1)],
                                    rhs=es_cols(kt, 256 * sl, 256),
                                    start=(kt == 0),
                                    stop=(kt == depth - 1),
                                )
                            nc.vector.tensor_mul(
                                out=axT[ds][:, 256 * sl : 256 * (sl + 1)],
                                in0=ps[:],
                                in1=rcb[sl][:],
                            )

                    # output projection per 128-query chunk
                    for sl in range(2):
                        for qq in range(2):
                            c0 = 256 * sl + P * qq
                            for ot in range(2):
                                pso = pso_pool.tile([P, 512], F32, name="ps_o")
                                for dc in range(DC):
                                    nc.tensor.matmul(
                                        pso[:],
                                        lhsT=axT[dc][:, c0 : c0 + P],
                                        rhs=wv_s[:, dc, 512 * ot : 512 * (ot + 1)],
                                        start=(dc == 0),
                                        stop=(dc == DC - 1),
                                    )
                                last = p == 1 and sl == 1 and qq == 1 and ot == 1
                                nh = 2 if last else 1  # split final copy+DMA
                                for h in range(nh):
                                    hw = 512 // nh
                                    ob = obpool.tile([P, hw], F32, name="ob")
                                    nc.scalar.activation(
                                        ob[:], pso[:, h * hw : (h + 1) * hw], AF.Copy
                                    )
                                    nc.sync.dma_start(
                                        out_r[
                                            :,
                                            (2 * p + sl) * 2 + qq,
                                            512 * ot + h * hw : 512 * ot
                                            + (h + 1) * hw,
                                        ],
                                        ob[:],
                                    )

    nc.compile()
    if not nc.is_finalized():
        nc.finalize()
    return nc


def _build_masks(fold: int) -> tuple[np.ndarray, np.ndarray]:
    """0/1 causal masks. Full tiles: [N_M512, 128, 512] (both slots of a
    pair); narrow tiles: [N_M256, 128, 256] (deep slot's tail kt)."""
    ki = np.arange(P)[:, None]
    qi = np.arange(QB)[None, :]
    t512, t256 = [], []
    for p, (dlo, dhi) in enumerate(PAIRS):
        b_lo = FOLD_QBLOCKS[fold][2 * p]
        b_hi = FOLD_QBLOCKS[fold][2 * p + 1]
        for kt in range(dlo):
            k0 = kt * P
            halves = [
                ((b * QB + qi) >= (k0 + ki)).astype(np.float32)
                for b in (b_lo, b_hi)
            ]
            t512.append(np.concatenate(halves, axis=1))
        for kt in range(dlo, dhi):
            k0 = kt * P
            t256.append(((b_hi * QB + qi) >= (k0 + ki)).astype(np.float32))
    bf = ml_dtypes.bfloat16
    return (
        np.ascontiguousarray(np.stack(t512).astype(bf)),
        np.ascontiguousarray(np.stack(t256).astype(bf)),
    )


def build_in_maps(inputs):
    x = np.asarray(inputs["inputs"], dtype=np.float32)
    bf = ml_dtypes.bfloat16
    wq = np.asarray(inputs["Wq"], dtype=np.float32)
    wk = np.asarray(inputs["Wk"], dtype=np.float32)
    m = np.ascontiguousarray((wq.T @ wk).astype(bf))  # [d_in, d_in]
    wvT = np.ascontiguousarray(np.asarray(inputs["Wv"], dtype=np.float32).T.astype(bf))

    masks = {f: _build_masks(f) for f in (0, 1)}
    in_maps = []
    for c in range(N_CORES):
        b, f = c // 2, c % 2
        xT = np.ascontiguousarray(x[b].T.astype(bf))  # [D, S]
        xn = np.ascontiguousarray(x[b].astype(bf))  # [S, D]
        xqT = np.ascontiguousarray(
            np.concatenate(
                [xT[:, qb * QB : (qb + 1) * QB] for qb in FOLD_QBLOCKS[f]], axis=1
            )
        )
        in_maps.append(
            {
                "m": m,
                "xqT": xqT,
                "xT": xT,
                "xn": xn,
                "wvT": wvT,
                "m512": masks[f][0],
                "m256": masks[f][1],
            }
        )
    return in_maps


def kernel(**inputs: np.ndarray) -> np.ndarray:
    in_maps = build_in_maps(inputs)
    nc = _build_nc()
    res = run_bass_kernel_spmd(nc, in_maps, core_ids=list(range(N_CORES)))

    out = np.empty((B, S, D), dtype=np.float32)
    for c in range(N_CORES):
        b, f = c // 2, c % 2
        o = res.results[c]["out"]  # [1024, 1024] rows in slot order
        for s, qb in enumerate(FOLD_QBLOCKS[f]):
            out[b, qb * QB : (qb + 1) * QB, :] = o[s * QB : (s + 1) * QB, :]
    return out


# revision 26
# speedup vs baseline: 1.2082x; 1.2082x over previous
"""Causal attention (B=4, S=2048, D=1024) on 8 Trainium2 NeuronCores.

Sharding: 2 cores per batch element, query blocks of 256 rows split by parity
(fold 0 takes odd blocks, fold 1 even) so causal work balances.

Algebraic restructure vs the straightforward QKV pipeline: with
M = Wq^T Wk (precomputed host-side from the weights), scores are
S = (x_q M) x^T, so no K projection is needed on-device and "K^T" is the raw
transposed input x^T already resident in SBUF. On the output side,
out = A (x Wv^T) is re-associated as (A x) Wv^T, so no V projection either:
the attention matrix contracts against raw x (natural layout), and one final
d x d projection by Wv^T produces the output. This removes the duplicated
K/V projections entirely (they were recomputed on both cores of a pair) at
zero communication cost.

All matmuls run in bf16 (fp32 PSUM accumulate). Scores are computed
transposed (S^T = (x^T)^T-stationary @ q'^T) so exp(S^T) tiles feed the
A-x contraction directly as the moving operand, producing ax^T =
sum_k x[k,:]^T es[k,:] in [d, q] layout, which in turn is the stationary for
the final projection out[q, o] = sum_d ax^T[d, q] Wv^T[d, o] -- every tensor
lands in its natural layout with no on-chip transposes.

Causal structure: per core 4 query slots of 256 rows; slot pairs (0,1) and
(2,3) share score passes. Static kt depths per pair are (4,8) and (12,16)
(fold-0 depths; fold 1 true depths are smaller and handled by its 0/1 masks,
which also zero the diagonal/overcomputed regions and keep softmax
denominators exact). Scores/es run 512 wide for kt < d_lo (both slots) and
256 wide for the deep slot's tail. Softmax skips max-subtraction (scaled
scores are ~N(0,1); exp cannot overflow), denominators via ones-column
matmuls per 128-query chunk.
"""

import sys

sys.path.insert(0, "/opt/trn_rl_repo")

import ml_dtypes
import numpy as np

import concourse.bass as bass  # noqa: F401
import concourse.mybir as mybir
import concourse.tile as tile
from concourse import bacc
from concourse.bass_utils import run_bass_kernel_spmd

F32 = mybir.dt.float32
BF16 = mybir.dt.bfloat16
AF = mybir.ActivationFunctionType

B, S, D = 4, 2048, 1024
P = 128
DC = D // P  # 8 contraction chunks
TC = S // P  # 16 context chunks
N_CORES = 8
SLOTS = 4
QB = 256
FOLD_QBLOCKS = {0: [1, 3, 5, 7], 1: [0, 2, 4, 6]}
# Static (fold-0) kt depths for slot pairs (0,1) and (2,3).
PAIRS = [(4, 8), (12, 16)]
N_M512 = sum(dlo for dlo, _ in PAIRS)  # full-width mask tiles
N_M256 = sum(dhi - dlo for dlo, dhi in PAIRS)  # deep-tail mask tiles
SCALE = 1.0 / np.sqrt(np.float32(D))
WARMUP_MM = 6  # dummy matmuls to release the HAM clock gate early


def _build_nc(repeat: int = 1):
    nc = bacc.Bacc("TRN2", target_bir_lowering=False, debug=False, num_devices=N_CORES)

    m_d = nc.declare_dram_parameter("m", [D, D], BF16, isOutput=False)
    xqT_d = nc.declare_dram_parameter("xqT", [D, SLOTS * QB], BF16, isOutput=False)
    xT_d = nc.declare_dram_parameter("xT", [D, S], BF16, isOutput=False)
    xn_d = nc.declare_dram_parameter("xn", [S, D], BF16, isOutput=False)
    wvT_d = nc.declare_dram_parameter("wvT", [D, D], BF16, isOutput=False)
    m512_d = nc.declare_dram_parameter("m512", [N_M512, P, 512], BF16, isOutput=False)
    m256_d = nc.declare_dram_parameter("m256", [N_M256, P, 256], BF16, isOutput=False)
    out_d = nc.declare_dram_parameter("out", [SLOTS * QB, D], F32, isOutput=True)

    m_r = m_d[:].rearrange("(ic p) j -> p ic j", p=P)  # [128, 8, 1024]
    xqT_r = xqT_d[:].rearrange("(ic p) q -> p ic q", p=P)  # [128, 8, 1024]
    xT_r = xT_d[:].rearrange("(dc p) t -> p dc t", p=P)  # [128, 8, 2048]
    xn_r = xn_d[:].rearrange("(tc p) d -> p tc d", p=P)  # [128, 16, 1024]
    wvT_r = wvT_d[:].rearrange("(dc p) o -> p dc o", p=P)
    m512_r = m512_d[:].rearrange("n p w -> p n w")  # [128, 16, 512]
    m256_r = m256_d[:].rearrange("n p w -> p n w")  # [128, 8, 256]
    out_r = out_d[:].rearrange("(qc p) o -> p qc o", p=P)  # [128, 8, 1024]

    with tile.TileContext(nc, pool_alloc_mode="queue") as tc:
      for _rep in range(repeat):
        with tc.tile_pool(name="resident", bufs=1) as res_pool:
            xT_s = res_pool.tile([P, DC, S], BF16, name="xT_s")
            xn_s = res_pool.tile([P, TC, D], BF16, name="xn_s")
            # q'^T as separate (d-slice, q-half) tiles so consumers wait on
            # exactly the chunk they read, not the whole tensor.
            qpT = {
                (ds, qt): res_pool.tile([P, 512], BF16, name=f"qpT_{ds}_{qt}")
                for ds in range(DC)
                for qt in range(2)
            }
            mk512 = res_pool.tile([P, N_M512, 512], BF16, name="mk512")
            mk256 = res_pool.tile([P, N_M256, 256], BF16, name="mk256")
            ones128 = res_pool.tile([P, P], BF16, name="ones128")
            scrap = res_pool.tile([P, 512], F32, name="scrap")
            warm_rhs = res_pool.tile([P, 512], BF16, name="warm_rhs")
            nc.vector.memset(ones128[:], 1.0)
            nc.vector.memset(warm_rhs[:], 0.0)

            # ---- Phase Q': q'^T = M^T.T @ xq^T -> qpT (SBUF) ---------------
            with (
                tc.tile_pool(name="m_pool", bufs=1) as mpool,
                tc.tile_pool(name="xq_pool", bufs=1) as xqpool,
                tc.tile_pool(name="psum_q", bufs=6, space="PSUM") as psq,
                tc.tile_pool(name="psum_w", bufs=1, space="PSUM") as psw,
            ):
                m_s = mpool.tile([P, DC, D], BF16, name="m_s")
                xq_s = xqpool.tile([P, DC, SLOTS * QB], BF16, name="xq_s")
                # Warm-up: the PE clock gate (HAM) starts at 1.2 GHz and only
                # reaches 2.4 GHz after ~3.4us of sustained activity. Run a
                # dummy chain during the initial DMA prefix so the real
                # matmuls start warm.
                if WARMUP_MM:
                    ps_w = psw.tile([P, 512], F32, name="ps_warm")
                    for i in range(WARMUP_MM):
                        nc.tensor.matmul(
                            ps_w[:],
                            lhsT=ones128[:],
                            rhs=warm_rhs[:],
                            start=(i == 0),
                            stop=(i == WARMUP_MM - 1),
                        )
                    nc.vector.tensor_copy(scrap[:], ps_w[:])
                # All loads on ONE queue, in critical-path priority order:
                # the HBM pipe is the bottleneck, so later-needed tensors
                # must not steal bandwidth from the q' operands.
                for ic in range(DC):
                    nc.sync.dma_start(m_s[:, ic, :], m_r[:, ic, :])
                    nc.sync.dma_start(xq_s[:, ic, :], xqT_r[:, ic, :])
                for dc in range(DC):
                    nc.sync.dma_start(xT_s[:, dc, :], xT_r[:, dc, :])
                nc.sync.dma_start(mk512[:], m512_r)
                for tc_i in range(TC // 2):
                    nc.sync.dma_start(xn_s[:, tc_i, :], xn_r[:, tc_i, :])
                nc.sync.dma_start(mk256[:], m256_r)
                for tc_i in range(TC // 2, TC):
                    nc.sync.dma_start(xn_s[:, tc_i, :], xn_r[:, tc_i, :])
                for qt in range(2):
                    for ds in range(DC):
                        ps = psq.tile([P, 512], F32, name="ps_q")
                        for ic in range(DC):
                            nc.tensor.matmul(
                                ps[:],
                                lhsT=m_s[:, ic, P * ds : P * (ds + 1)],
                                rhs=xq_s[:, ic, 512 * qt : 512 * (qt + 1)],
                                start=(ic == 0),
                                stop=(ic == DC - 1),
                            )
                        nc.vector.tensor_copy(qpT[(ds, qt)][:], ps[:])

            # ---- Attention: scores -> exp/mask -> ax^T -> out projection ---
            with (
                tc.tile_pool(name="wv_pool", bufs=1) as wvpool,
                tc.tile_pool(name="es512_pool", bufs=14) as e5pool,
                tc.tile_pool(name="es256_pool", bufs=6) as e2pool,
                tc.tile_pool(name="ax_pool", bufs=2) as axpool,
                tc.tile_pool(name="ob_pool", bufs=3) as obpool,
                tc.tile_pool(name="rc_pool", bufs=2) as rcpool,
                tc.tile_pool(name="rcb_pool", bufs=2) as rcbpool,
                tc.tile_pool(name="psum_s", bufs=2, space="PSUM") as pss,
                tc.tile_pool(name="psum_a", bufs=2, space="PSUM") as psa,
                tc.tile_pool(name="psum_d", bufs=2, space="PSUM") as psd_pool,
                tc.tile_pool(name="psum_o", bufs=2, space="PSUM") as pso_pool,
            ):
                wv_s = wvpool.tile([P, DC, D], BF16, name="wv_s")
                for dc in range(DC):
                    nc.sync.dma_start(wv_s[:, dc, :], wvT_r[:, dc, :])

                i512 = 0
                i256 = 0
                for p, (dlo, dhi) in enumerate(PAIRS):
                    # scores + exp + mask over the pair's static depth
                    es_full = []
                    es_nar = []
                    for kt in range(dhi):
                        full = kt < dlo
                        w = 512 if full else 256
                        c0 = 0 if full else 256
                        ps = pss.tile([P, w], F32, name="ps_s")
                        for dc in range(DC):
                            nc.tensor.matmul(
                                ps[:],
                                lhsT=xT_s[:, dc, P * kt : P * (kt + 1)],
                                rhs=qpT[(dc, p)][:, c0 : c0 + w],
                                start=(dc == 0),
                                stop=(dc == DC - 1),
                            )
                        pool = e5pool if full else e2pool
                        es = pool.tile([P, w], BF16, name="es")
                        nc.scalar.activation(es[:], ps[:], AF.Exp, scale=SCALE)
                        if full:
                            mk = mk512[:, i512, :]
                            i512 += 1
                            es_full.append(es)
                        else:
                            mk = mk256[:, i256, :]
                            i256 += 1
                            es_nar.append(es)
                        nc.vector.tensor_mul(out=es[:], in0=es[:], in1=mk)

                    def es_cols(kt, c0, w, dlo=dlo, es_full=es_full, es_nar=es_nar):
                        """es slice for pair-local cols [c0, c0+w)."""
                        if kt < dlo:
                            return es_full[kt][:, c0 : c0 + w]
                        assert c0 >= 256
                        return es_nar[kt - dlo][:, c0 - 256 : c0 - 256 + w]

                    # denominators: all-ones [128,128] stationary replicates
                    # denom[q] = sum_k es[k, q] across every partition, so
                    # the reciprocal runs partition-parallel and the scale
                    # fuses into the ax^T PSUM->SBUF copy below.
                    rcb = {}
                    for sl, depth in ((0, dlo), (1, dhi)):
                        psd = psd_pool.tile([P, QB], F32, name="ps_d")
                        for kt in range(depth):
                            nc.tensor.matmul(
                                psd[:],
                                lhsT=ones128[:],
                                rhs=es_cols(kt, 256 * sl, 256),
                                start=(kt == 0),
                                stop=(kt == depth - 1),
                            )
                        rcb[sl] = rcbpool.tile([P, QB], F32, name="rcb")
                        nc.vector.reciprocal(rcb[sl][:], psd[:])

                    # ax^T[d, q] = (sum_k x[k, d] es[k, q]) / denom[q]
                    axT = {
                        ds: axpool.tile([P, 512], BF16, name=f"axT_{ds}")
                        for ds in range(DC)
                    }
                    for ds in range(DC):
                        for sl, depth in ((0, dlo), (1, dhi)):
                            ps = psa.tile([P, QB], F32, name="ps_a")
                            for kt in range(depth):
                                nc.tensor.matmul(
                                    ps[:],
                                    lhsT=xn_s[:, kt, P * ds : P * (ds + 1)],
                                    rhs=es_cols(kt, 256 * sl, 256),
                                    start=(kt == 0),
                                    stop=(kt == depth - 1),
                                )
                            nc.vector.tensor_mul(
                                out=axT[ds][:, 256 * sl : 256 * (sl + 1)],
                                in0=ps[:],
                                in1=rcb[sl][:],
                            )

                    # output projection per 128-query chunk
                    for sl in range(2):
                        for qq in range(2):
                            c0 = 256 * sl + P * qq
                            for ot in range(2):
                                pso = pso_pool.tile([P, 512], F32, name="ps_o")
                                for dc in range(DC):
                                    nc.tensor.matmul(
                                        pso[:],
                                        lhsT=axT[dc][:, c0 : c0 + P],
                                        rhs=wv_s[:, dc, 512 * ot : 512 * (ot + 1)],
                                        start=(dc == 0),
                                        stop=(dc == DC - 1),
                                    )
                                last = p == 1 and sl == 1 and qq == 1 and ot == 1
                                nh = 2 if last else 1  # split final copy+DMA
                                for h in range(nh):
                                    hw = 512 // nh
                                    ob = obpool.tile([P, hw], F32, name="ob")
                                    nc.scalar.activation(
                                        ob[:], pso[:, h * hw : (h + 1) * hw], AF.Copy
                                    )
                                    nc.sync.dma_start(
                                        out_r[
                                            :,
                                            (2 * p + sl) * 2 + qq,
                                            512 * ot + h * hw : 512 * ot
                                            + (h + 1) * hw,
                                        ],
                                        ob[:],
                                    )

    nc.compile()
    if not nc.is_finalized():
        nc.finalize()
    return nc


def _build_masks(fold: int) -> tuple[np.ndarray, np.ndarray]:
    """0/1 causal masks. Full tiles: [N_M512, 128, 512] (both slots of a
    pair); narrow tiles: [N_M256, 128, 256] (deep slot's tail kt)."""
    ki = np.arange(P)[:, None]
    qi = np.arange(QB)[None, :]
    t512, t256 = [], []
    for p, (dlo, dhi) in enumerate(PAIRS):
        b_lo = FOLD_QBLOCKS[fold][2 * p]
        b_hi = FOLD_QBLOCKS[fold][2 * p + 1]
        for kt in range(dlo):
            k0 = kt * P
            halves = [
                ((b * QB + qi) >= (k0 + ki)).astype(np.float32)
                for b in (b_lo, b_hi)
            ]
            t512.append(np.concatenate(halves, axis=1))
        for kt in range(dlo, dhi):
            k0 = kt * P
            t256.append(((b_hi * QB + qi) >= (k0 + ki)).astype(np.float32))
    bf = ml_dtypes.bfloat16
    return (
        np.ascontiguousarray(np.stack(t512).astype(bf)),
        np.ascontiguousarray(np.stack(t256).astype(bf)),
    )


def build_in_maps(inputs):
    x = np.asarray(inputs["inputs"], dtype=np.float32)
    bf = ml_dtypes.bfloat16
    wq = np.asarray(inputs["Wq"], dtype=np.float32)
    wk = np.asarray(inputs["Wk"], dtype=np.float32)
    m = np.ascontiguousarray((wq.T @ wk).astype(bf))  # [d_in, d_in]
    wvT = np.ascontiguousarray(np.asarray(inputs["Wv"], dtype=np.float32).T.astype(bf))

    masks = {f: _build_masks(f) for f in (0, 1)}
    in_maps = []
    for c in range(N_CORES):
        b, f = c // 2, c % 2
        xT = np.ascontiguousarray(x[b].T.astype(bf))  # [D, S]
        xn = np.ascontiguousarray(x[b].astype(bf))  # [S, D]
        xqT = np.ascontiguousarray(
            np.concatenate(
                [xT[:, qb * QB : (qb + 1) * QB] for qb in FOLD_QBLOCKS[f]], axis=1
            )
        )
        in_maps.append(
            {
                "m": m,
                "xqT": xqT,
                "xT": xT,
                "xn": xn,
                "wvT": wvT,
                "m512": masks[f][0],
                "m256": masks[f][1],
            }
        )
    return in_maps


def kernel(**inputs: np.ndarray) -> np.ndarray:
    in_maps = build_in_maps(inputs)
    nc = _build_nc()
    res = run_bass_kernel_spmd(nc, in_maps, core_ids=list(range(N_CORES)))

    out = np.empty((B, S, D), dtype=np.float32)
    for c in range(N_CORES):
        b, f = c // 2, c % 2
        o = res.results[c]["out"]  # [1024, 1024] rows in slot order
        for s, qb in enumerate(FOLD_QBLOCKS[f]):
            out[b, qb * QB : (qb + 1) * QB, :] = o[s * QB : (s + 1) * QB, :]
    return out
